# revision 8
# baseline (speedup 1.0000x reference)
"""Haar DWT (2x2 block transform) for Trainium2, data-parallel over 8 NeuronCores.

Full input x: (16, 64, 256, 256) fp32 -> output (16, 256, 128, 128) fp32 where
out[b, 4c+k] = subband k of channel c, k in [cA, cH, cV, cD].

Sharding: batch dim 16 -> 2 per core. Per core the (2, 64) batch/channel dims
flatten to exactly 128 images = the SBUF partition dim; each partition owns one
256x256 image laid out contiguously in its free dim.

The op is memory-bound and the correctness gate (rel err < 2e-2) leaves room
for bf16 I/O (total rounding ~3*2^-9 rel, measured ~7e-3). The host converts
the input to bf16 once, folding in the Haar 1/2 (exact pow2 scale) and
de-interleaving even/odd columns per row, so the device moves 16 MiB in +
16 MiB out per core (vs 32+32 in fp32) and every DVE operand is a unit-stride
bf16 run -- the 2x_1P packed mode applies to all 8 butterfly ops per tile
(~71 us DVE busy vs ~141 us for the fp32 pipeline).

Per-core pipeline (per row-tile of every image; row = [128 even | 128 odd]):
  1. DMA in  (128, K) bf16 -> xb               [nc.sync / SP HWDGE ring]
  2. VectorE: u_e=te+be, u_o=to+bo, v_e=be-te, v_o=bo-to  [vertical butterfly]
  3. VectorE: cA=u_e+u_o, cH=v_e+v_o, cV=u_o-u_e, cD=v_o-v_e [horizontal]
  4. DMA out (128, 4 x K/4) bf16 -> 4 subband regions [nc.scalar / ACT ring]

Stores ride the ACT HWDGE ring: the issuing engine blocks on each store's
sem wait (DVE pass 2), which is harmless because ACT has no other work, and
it keeps the SP ring pure loads (rings are FIFO per engine). Loads and
stores are 16 MiB each, so the two rings' SDMA packet round-robin is
balanced. First/last tiles are 16 rows to start compute sooner and shorten
the drain tail.
"""

import numpy as np

B, C, H, W = 16, 64, 256, 256
N_CORES = 8
B_PER = B // N_CORES  # 2
IMGS = B_PER * C  # 128 images/core = SBUF partitions
IMG_PIX = H * W  # 65536 elements per image
TILE_ROWS = [16, 32, 32, 32, 32, 32, 32, 32, 16]
assert sum(TILE_ROWS) == H
MAX_K = max(TILE_ROWS) * W  # slot size for the tile pools (elems)
SUB = (H // 2) * (W // 2)  # 16384 elements per subband

_CACHE: dict = {}


def build_nc():
    import concourse.bacc as bacc
    import concourse.mybir as mybir
    from concourse.tile import TileContext

    bf16 = mybir.dt.bfloat16
    # Bacc (not plain Bass): its generate_event_semaphores pass splits
    # multi-sem waits, which the TRN2 static-DMA encoding can't hold.
    nc = bacc.Bacc(target_bir_lowering=False, debug=False)
    x = nc.dram_tensor("x", [IMGS, IMG_PIX], bf16, kind="ExternalInput")
    y = nc.dram_tensor("y", [IMGS, 4 * SUB], bf16, kind="ExternalOutput")
    # y viewed per subband: (128, 4, 16384)
    y_sub = y[:].rearrange("p (k s) -> p k s", k=4)

    with TileContext(nc) as tc:
        with (
            tc.tile_pool(name="xb", bufs=4) as pool_xb,
            tc.tile_pool(name="uv", bufs=2) as pool_uv,
            tc.tile_pool(name="res", bufs=4) as pool_res,
        ):
            row0 = 0
            for rows in TILE_ROWS:
                K = rows * W  # free elems / partition this tile
                q = K // 4  # elems per quarter (= per subband) this tile
                hw = W // 2  # 128: row length after the even/odd split
                xb = pool_xb.tile([IMGS, MAX_K], bf16)
                nc.sync.dma_start(
                    out=xb[:, 0:K], in_=x[:, row0 * W : row0 * W + K]
                )

                # vertical butterfly on row pairs (2i, 2i+1); each input row is
                # [128 even cols | 128 odd cols], so every operand is a
                # unit-stride 128-elem bf16 run -> DVE 2x packed mode
                xv = xb[:, 0:K].rearrange(
                    "p (i two eo w) -> p i two eo w", two=2, eo=2, w=hw
                )
                te, to = xv[:, :, 0, 0], xv[:, :, 0, 1]  # row 2i:   a, b
                be, bo = xv[:, :, 1, 0], xv[:, :, 1, 1]  # row 2i+1: c, d
                uv = pool_uv.tile([IMGS, MAX_K], bf16)
                uvq = uv[:, 0:K].rearrange("p (k i w) -> p k i w", k=4, w=hw)
                u_e, u_o, v_e, v_o = (uvq[:, k] for k in range(4))
                nc.vector.tensor_add(out=u_e, in0=te, in1=be)  # a+c
                nc.vector.tensor_add(out=u_o, in0=to, in1=bo)  # b+d
                nc.vector.tensor_sub(out=v_e, in0=be, in1=te)  # c-a
                nc.vector.tensor_sub(out=v_o, in0=bo, in1=to)  # d-b

                # horizontal butterfly: plain contiguous q-elem arrays
                res = pool_res.tile([IMGS, MAX_K], bf16)
                ue_f, uo_f = uv[:, 0:q], uv[:, q : 2 * q]
                ve_f, vo_f = uv[:, 2 * q : 3 * q], uv[:, 3 * q : 4 * q]
                nc.vector.tensor_add(out=res[:, 0:q], in0=ue_f, in1=uo_f)  # cA
                nc.vector.tensor_add(out=res[:, q : 2 * q], in0=ve_f, in1=vo_f)  # cH
                nc.vector.tensor_sub(out=res[:, 2 * q : 3 * q], in0=uo_f, in1=ue_f)  # cV
                # cD rides the (otherwise idle) GPSIMD engine: measured 66 G
                # elem/s vs DVE's 236, so one of the 8 butterfly ops fits with
                # slack and takes DVE (the pacer) from 8 to 7 ops/tile. cD is a
                # pass-2 leaf: no DVE op consumes it, the store just waits on
                # both engines (bacc splits the multi-sem wait).
                nc.gpsimd.tensor_sub(out=res[:, 3 * q : 4 * q], in0=vo_f, in1=ve_f)  # cD

                # res = [cA|cH|cV|cD]; one strided store to all 4 subband regions
                o0 = (row0 // 2) * hw  # out offset within each subband
                dst = y_sub[:, :, o0 : o0 + q]  # (128, 4, q)
                src = res[:, 0:K].rearrange("p (k o) -> p k o", k=4)
                nc.scalar.dma_start(out=dst, in_=src)
                row0 += rows
    # run Bacc's pass pipeline (regalloc, DCE, event-semaphore wait splitting)
    nc.compile()
    return nc


def _get_nc():
    if "nc" not in _CACHE:
        _CACHE["nc"] = build_nc()
    return _CACHE["nc"]


def _prep_input(x: np.ndarray) -> np.ndarray:
    """fp32 (B,C,H,W) -> bf16 (B,C,H,W) with 0.5 folded in and each row
    de-interleaved to [even cols | odd cols]."""
    import ml_dtypes

    xr = x.reshape(B, C, H, W // 2, 2).transpose(0, 1, 2, 4, 3)
    return np.ascontiguousarray(
        (xr * np.float32(0.5)).astype(ml_dtypes.bfloat16)
    ).reshape(B, C, H * W)


def _unshard(results):
    return np.concatenate(
        [
            np.asarray(r["y"]).astype(np.float32).reshape(B_PER, C * 4, H // 2, W // 2)
            for r in results
        ],
        axis=0,
    )


def kernel(x: np.ndarray) -> np.ndarray:
    from concourse.bass_utils import run_bass_kernel_spmd

    x = np.asarray(x)
    assert x.shape == (B, C, H, W), x.shape
    xh = _prep_input(np.ascontiguousarray(x, dtype=np.float32))

    nc = _get_nc()
    in_maps = [
        {"x": xh[c * B_PER : (c + 1) * B_PER].reshape(IMGS, IMG_PIX)}
        for c in range(N_CORES)
    ]
    results = run_bass_kernel_spmd(nc, in_maps, core_ids=list(range(N_CORES))).results
    return _unshard(results)


# revision 9
# speedup vs baseline: 1.0938x; 1.0938x over previous
"""Haar DWT (2x2 block transform) for Trainium2, data-parallel over 8 NeuronCores.

Full input x: (16, 64, 256, 256) fp32 -> output (16, 256, 128, 128) fp32 where
out[b, 4c+k] = subband k of channel c, k in [cA, cH, cV, cD].

Sharding: batch dim 16 -> 2 per core. Per core the (2, 64) batch/channel dims
flatten to exactly 128 images = the SBUF partition dim; each partition owns one
256x256 image laid out contiguously in its free dim.

The op is memory-bound and the correctness gate (rel err < 2e-2) leaves room
for bf16 I/O (total rounding ~3*2^-9 rel, measured ~7e-3). The host converts
the input to bf16 once, folding in the Haar 1/2 (exact pow2 scale) and
de-interleaving even/odd columns per row, so the device moves 16 MiB in +
16 MiB out per core (vs 32+32 in fp32) and every DVE operand is a unit-stride
bf16 run -- the 2x_1P packed mode applies to all 8 butterfly ops per tile
(~71 us DVE busy vs ~141 us for the fp32 pipeline).

Per-core pipeline (per row-tile of every image; row = [128 even | 128 odd]):
  1. DMA in  (128, K) bf16 -> xb               [nc.sync / SP HWDGE ring]
  2. VectorE: u_e=te+be, u_o=to+bo, v_e=be-te, v_o=bo-to  [vertical butterfly]
  3. VectorE: cA=u_e+u_o, cH=v_e+v_o, cV=u_o-u_e, cD=v_o-v_e [horizontal]
  4. DMA out (128, 4 x K/4) bf16 -> 4 subband regions [nc.scalar / ACT ring]

Stores ride the ACT HWDGE ring: the issuing engine blocks on each store's
sem wait (DVE pass 2), which is harmless because ACT has no other work, and
it keeps the SP ring pure loads (rings are FIFO per engine). Loads and
stores are 16 MiB each, so the two rings' SDMA packet round-robin is
balanced. First/last tiles are 16 rows to start compute sooner and shorten
the drain tail.
"""

import numpy as np

B, C, H, W = 16, 64, 256, 256
N_CORES = 8
B_PER = B // N_CORES  # 2
IMGS = B_PER * C  # 128 images/core = SBUF partitions
IMG_PIX = H * W  # 65536 elements per image
TILE_ROWS = [8, 24, 32, 32, 32, 32, 32, 32, 24, 8]
assert sum(TILE_ROWS) == H
MAX_K = max(TILE_ROWS) * W  # slot size for the tile pools (elems)
SUB = (H // 2) * (W // 2)  # 16384 elements per subband

_CACHE: dict = {}


def build_nc():
    import concourse.bacc as bacc
    import concourse.mybir as mybir
    from concourse.tile import TileContext

    bf16 = mybir.dt.bfloat16
    # Bacc (not plain Bass): its generate_event_semaphores pass splits
    # multi-sem waits, which the TRN2 static-DMA encoding can't hold.
    nc = bacc.Bacc(target_bir_lowering=False, debug=False)
    x = nc.dram_tensor("x", [IMGS, IMG_PIX], bf16, kind="ExternalInput")
    y = nc.dram_tensor("y", [IMGS, 4 * SUB], bf16, kind="ExternalOutput")
    # y viewed per subband: (128, 4, 16384)
    y_sub = y[:].rearrange("p (k s) -> p k s", k=4)

    with TileContext(nc) as tc:
        with (
            tc.tile_pool(name="xb", bufs=4) as pool_xb,
            tc.tile_pool(name="uv", bufs=3) as pool_uv,
            tc.tile_pool(name="res", bufs=4) as pool_res,
        ):
            row0 = 0
            for rows in TILE_ROWS:
                K = rows * W  # free elems / partition this tile
                q = K // 4  # elems per quarter (= per subband) this tile
                hw = W // 2  # 128: row length after the even/odd split
                xb = pool_xb.tile([IMGS, MAX_K], bf16)
                nc.sync.dma_start(
                    out=xb[:, 0:K], in_=x[:, row0 * W : row0 * W + K]
                )

                # vertical butterfly on row pairs (2i, 2i+1); each input row is
                # [128 even cols | 128 odd cols], so every operand is a
                # unit-stride 128-elem bf16 run -> DVE 2x packed mode
                xv = xb[:, 0:K].rearrange(
                    "p (i two eo w) -> p i two eo w", two=2, eo=2, w=hw
                )
                te, to = xv[:, :, 0, 0], xv[:, :, 0, 1]  # row 2i:   a, b
                be, bo = xv[:, :, 1, 0], xv[:, :, 1, 1]  # row 2i+1: c, d
                uv = pool_uv.tile([IMGS, MAX_K], bf16)
                uvq = uv[:, 0:K].rearrange("p (k i w) -> p k i w", k=4, w=hw)
                u_e, u_o, v_e, v_o = (uvq[:, k] for k in range(4))
                nc.vector.tensor_add(out=u_e, in0=te, in1=be)  # a+c
                nc.vector.tensor_add(out=u_o, in0=to, in1=bo)  # b+d
                nc.vector.tensor_sub(out=v_e, in0=be, in1=te)  # c-a
                nc.vector.tensor_sub(out=v_o, in0=bo, in1=to)  # d-b

                # horizontal butterfly: plain contiguous q-elem arrays
                res = pool_res.tile([IMGS, MAX_K], bf16)
                ue_f, uo_f = uv[:, 0:q], uv[:, q : 2 * q]
                ve_f, vo_f = uv[:, 2 * q : 3 * q], uv[:, 3 * q : 4 * q]
                nc.vector.tensor_add(out=res[:, 0:q], in0=ue_f, in1=uo_f)  # cA
                nc.vector.tensor_add(out=res[:, q : 2 * q], in0=ve_f, in1=vo_f)  # cH
                nc.vector.tensor_sub(out=res[:, 2 * q : 3 * q], in0=uo_f, in1=ue_f)  # cV
                nc.vector.tensor_sub(out=res[:, 3 * q : 4 * q], in0=vo_f, in1=ve_f)  # cD

                # res = [cA|cH|cV|cD]; one strided store to all 4 subband regions
                o0 = (row0 // 2) * hw  # out offset within each subband
                dst = y_sub[:, :, o0 : o0 + q]  # (128, 4, q)
                src = res[:, 0:K].rearrange("p (k o) -> p k o", k=4)
                nc.scalar.dma_start(out=dst, in_=src)
                row0 += rows
    # run Bacc's pass pipeline (regalloc, DCE, event-semaphore wait splitting)
    nc.compile()
    return nc


def _get_nc():
    if "nc" not in _CACHE:
        _CACHE["nc"] = build_nc()
    return _CACHE["nc"]


def _prep_input(x: np.ndarray) -> np.ndarray:
    """fp32 (B,C,H,W) -> bf16 (B,C,H,W) with 0.5 folded in and each row
    de-interleaved to [even cols | odd cols]."""
    import ml_dtypes

    xr = x.reshape(B, C, H, W // 2, 2).transpose(0, 1, 2, 4, 3)
    return np.ascontiguousarray(
        (xr * np.float32(0.5)).astype(ml_dtypes.bfloat16)
    ).reshape(B, C, H * W)


def _unshard(results):
    return np.concatenate(
        [
            np.asarray(r["y"]).astype(np.float32).reshape(B_PER, C * 4, H // 2, W // 2)
            for r in results
        ],
        axis=0,
    )


def kernel(x: np.ndarray) -> np.ndarray:
    from concourse.bass_utils import run_bass_kernel_spmd

    x = np.asarray(x)
    assert x.shape == (B, C, H, W), x.shape
    xh = _prep_input(np.ascontiguousarray(x, dtype=np.float32))

    nc = _get_nc()
    in_maps = [
        {"x": xh[c * B_PER : (c + 1) * B_PER].reshape(IMGS, IMG_PIX)}
        for c in range(N_CORES)
    ]
    results = run_bass_kernel_spmd(nc, in_maps, core_ids=list(range(N_CORES))).results
    return _unshard(results)
